# revision 22
# baseline (speedup 1.0000x reference)
"""Trainium2 Bass kernel for nn_BoundaryLoss (3D-Laplacian boundary loss).

reference semantics (fp32):
    probs = softmax(logits, axis=1)[:, 1:]                  # (B, C-1, D, H, W)
    tmask = one_hot(targets)[classes 1..C-1]                # (B, C-1, D, H, W)
    loss  = mean((|lap3(probs)| - |lap3(tmask)|)**2)        # lap3 = 6-neighbour
                                                            # Laplacian, zero pad

Distribution: pure data parallelism over H (256 rows -> 8 slices of 32 rows,
plus one halo row on each side).  Each core computes the squared-error sum
over its slice; the host adds the 8 partial sums and divides by the global
element count.

On-core layout: SBUF partitions = (b, d) = 2*64 = 128.  Free dim = (h, w') with
w' = W+2 (two zero "guard" columns per row so the w+-1 stencil shifts read
zeros across row boundaries).  Engine assignment (engine-balanced; the
baseline's gpsimd masks at 131us/op serialized the whole kernel):
  - ScalarE: exp (chunked for pipelining), |.| of the two laplacians
  - VectorE: softmax reciprocal (reciprocal_approx_fast), probs mult,
    one-hot masks (tensor_scalar is_equal), diff, fused square+reduce
  - TensorE: softmax denominator accumulation + the 7-point laplacians
    (d+-1/-6 via block-tridiagonal stationary; h/w shifts via offset
    identity-stationary matmuls), PSUM fp32 accumulation
"""

import numpy as np
import ml_dtypes

import concourse.bass as bass
import concourse.bacc as bacc
import concourse.tile as tile
from concourse import mybir
from concourse.bass_utils import run_bass_kernel_spmd

# Problem shape (hardcoded; harness contract)
B, C, D, H, W = 2, 4, 64, 256, 256
NCORES = 8
HS = H // NCORES        # 32 output rows per core
HL = HS + 2             # 34 input rows (1 halo row each side)
WG = W + 2              # guarded row stride in SBUF free dim
GROUP = 4               # output rows per PSUM group (2 banks)
NEG = -100.0            # pad value for classes 1..3 -> softmax prob ~ 0
NTOT = B * (C - 1) * D * H * W  # mean denominator

F32 = mybir.dt.float32
BF16 = mybir.dt.bfloat16
AX = mybir.AxisListType
OP = mybir.AluOpType
AF = mybir.ActivationFunctionType

N_SLOTS = 3 * (HS // GROUP)  # one accumulator slot per (class, group)
import os as _os
STAGE = int(_os.environ.get("K_STAGE", "5"))  # debug bisect: 1..5
# row chunks for exp / probs (pipelining: denominator groups only need their
# own rows, so the PE can start ~8 rows in instead of after 4 whole-class exps)
CHUNKS = [(0, 6), (6, 14), (14, 22), (22, 30), (30, 34)]


def _stationaries():
    """T_D: d-stencil (d+-1 within the same b, -6 on the diagonal) on the
    interleaved partition layout p = 2*d + b.  wI: identity.  Exact in bf16."""
    td = np.zeros((128, 128), dtype=np.float32)
    for p in range(128):
        td[p, p] = -6.0
        d, b = divmod(p, 2)
        if d > 0:
            td[p - 2, p] = 1.0
        if d < D - 1:
            td[p + 2, p] = 1.0
    ident = np.eye(128, dtype=np.float32)
    return (td.astype(ml_dtypes.bfloat16), ident.astype(ml_dtypes.bfloat16))


def _emit(tc):
    nc = tc.nc
    # host pre-interleaves to partition order p = 2*d + b, so every DMA is a
    # plain 2D full-partition transfer
    lg = nc.dram_tensor("logits", [C, 128, HL, WG], BF16, kind="ExternalInput").ap()
    tg = nc.dram_tensor("targets", [128, HL, WG], BF16, kind="ExternalInput").ap()
    wtd_d = nc.dram_tensor("wTD", [128, 128], BF16, kind="ExternalInput").ap()
    wi_d = nc.dram_tensor("wI", [128, 128], BF16, kind="ExternalInput").ap()
    out_d = nc.dram_tensor("out", [128, N_SLOTS], F32, kind="ExternalOutput").ap()

    with (
        tc.tile_pool(name="singles", bufs=1) as singles,
        tc.tile_pool(name="mpool", bufs=3) as mpool,
        tc.tile_pool(name="rpool", bufs=2) as rpool,
        tc.tile_pool(name="tailpool", bufs=3) as tailpool,
        tc.tile_pool(name="psum", bufs=4, space="PSUM") as psum,
    ):
        # --- constants / persistent tiles ---
        wtd = singles.tile([128, 128], BF16, tag="wtd")
        wi = singles.tile([128, 128], BF16, tag="wi")
        nc.sync.dma_start(out=wtd, in_=wtd_d)
        nc.sync.dma_start(out=wi, in_=wi_d)
        slots = singles.tile([128, N_SLOTS], F32, tag="slots")
        nc.vector.memset(slots, 0.0)

        # --- exp of all 4 classes, chunked by rows ---
        # logits arrive bf16, guards 0.0 (-> e guard 1.0, killed by rbf=0),
        # halo rows 0.0 for class 0 / NEG for classes 1..3.
        # DMA + exp are chunked and interleaved class-major so the first
        # denominator matmuls can start ~2us in instead of after 4 whole-class
        # DMAs; exp is done in place on the DMA'd tile.
        # targets: host pads guards/halo with 255 so is_equal(c) vanishes
        # there; DMA chunks trail the logits chunks (mask laps start early)
        tgt = singles.tile([128, HL, WG], BF16, tag="tgt")
        e = []
        for ci in range(C):
            t = singles.tile([128, HL, WG], BF16, tag=f"e{ci}")
            e.append(t)
        for r0, r1 in CHUNKS:
            for ci in range(C):
                nc.sync.dma_start(
                    out=e[ci][:, r0:r1, :], in_=lg[ci][:, r0:r1, :]
                )
                nc.scalar.activation(
                    e[ci][:, r0:r1, :], e[ci][:, r0:r1, :], AF.Exp
                )
            nc.sync.dma_start(out=tgt[:, r0:r1, :], in_=tg[:, r0:r1, :])

        # --- denominator: pairwise class sums on DVE (e0 += e1 in place --
        # e0 is denominator-only -- and s23 = e2+e3 into scratch), then a
        # 2-pass PE accumulate instead of 4; recip per 4-row group.
        # Adds / denominator groups / recips are interleaved chunk by chunk
        # so the DVE recip chain is never queued behind all the adds. ---
        rbf = singles.tile([128, HL, WG], BF16, tag="rbf")
        nc.vector.memset(rbf[:, :, W:WG], 0.0)  # kills probs guards
        s23 = []

        def s23_rows(h0, j):
            for (r0, r1, sc) in s23:
                if r0 <= h0 + j and h0 + j + 2 <= r1:
                    return sc[:, h0 + j - r0:h0 + j - r0 + 2, 0:W]
            raise AssertionError("row pair spans chunks")

        done_h = 0
        for r0, r1 in CHUNKS if STAGE >= 2 else []:
            nc.vector.tensor_tensor(
                out=e[0][:, r0:r1, :], in0=e[0][:, r0:r1, :],
                in1=e[1][:, r0:r1, :], op=OP.add,
            )
            sc = rpool.tile([128, 8, WG], BF16, tag="s23")
            nc.vector.tensor_tensor(
                out=sc[:, 0:r1 - r0, :], in0=e[2][:, r0:r1, :],
                in1=e[3][:, r0:r1, :], op=OP.add,
            )
            s23.append((r0, r1, sc))
            while done_h < HL and done_h + min(GROUP, HL - done_h) <= r1:
                h0 = done_h
                nr = min(GROUP, HL - h0)
                done_h += nr
                st = psum.tile([128, GROUP, W], F32, tag="ps")
                for si in range(2):
                    for j in range(0, nr, 2):
                        rhs = (e[0][:, h0 + j:h0 + j + 2, 0:W] if si == 0
                               else s23_rows(h0, j))
                        nc.tensor.matmul(
                            out=st[:, j:j + 2, :],
                            lhsT=wi,
                            rhs=rhs,
                            start=(si == 0),
                            stop=(si == 1),
                        )
                sflat = st[:, 0:nr, :].rearrange("p h w -> p (h w)")
                rf = rpool.tile([128, GROUP * W], F32, tag="rf")
                nc.vector.reciprocal_approx_fast(
                    out=rf[:, 0:nr * W], in_=sflat
                )
                nc.vector.tensor_copy(
                    rbf[:, h0:h0 + nr, 0:W],
                    rf[:, 0:nr * W].rearrange("p (h w) -> p h w", w=W),
                )

        # --- probs (in place: e_c *= rbf, chunked) and masks, all classes
        # before the lap loops so the DVE work for class c+1 is never queued
        # behind class c's tail ops ---
        m = []
        for ci in range(1, C) if STAGE >= 3 else []:
            for r0, r1 in CHUNKS:
                nc.vector.tensor_tensor(
                    out=e[ci][:, r0:r1, :],
                    in0=e[ci][:, r0:r1, :],
                    in1=rbf[:, r0:r1, :],
                    op=OP.mult,
                )
            mc = mpool.tile([128, HL, WG], BF16, tag="m")
            for r0, r1 in CHUNKS:
                nc.vector.tensor_scalar(
                    mc[:, r0:r1, :], tgt[:, r0:r1, :], float(ci), None,
                    OP.is_equal,
                )
            m.append(mc)

        # --- laplacians (PE) + tail ---
        def lap_matmuls(pt, src, r0):
            """Accumulate the 7-point Laplacian of src rows [r0, r0+GROUP) into
            psum tile pt.  src is a [128, HL, WG] tile; output rows are
            r0..r0+3 (absolute row indices in the tile).  Each matmul reads a
            4-row window slice so Tile sees region deps (laps can start as
            soon as the producing probs/mask chunk lands, not the whole
            tile)."""
            passes = [
                (wtd, 0),       # d+-1 and -6 center
                (wi, -WG),      # h-1
                (wi, WG),       # h+1
                (wi, -1),       # w-1 (guards supply the zero pad)
                (wi, 1),        # w+1
            ]
            for pi, (wmat, sh) in enumerate(passes):
                for j in range(0, GROUP, 2):
                    w0 = r0 + j - 1
                    win = src[:, w0:w0 + 4, :].rearrange("p h w -> p (h w)")
                    off = WG + sh
                    rhs = win[:, off:off + 2 * WG].rearrange(
                        "p (h w) -> p h w", w=WG
                    )[:, :, 0:W]
                    nc.tensor.matmul(
                        out=pt[:, j:j + 2, :],
                        lhsT=wmat,
                        rhs=rhs,
                        start=(pi == 0),
                        stop=(pi == len(passes) - 1),
                    )

        for ci in range(1, C) if STAGE >= 4 else []:
            pc = e[ci]
            mc = m[ci - 1]
            for g in range(HS // GROUP):
                r0 = 1 + g * GROUP
                pp = psum.tile([128, GROUP, W], F32, tag="ps")
                pt = psum.tile([128, GROUP, W], F32, tag="ps")
                lap_matmuls(pp, pc, r0)
                lap_matmuls(pt, mc, r0)
                a = tailpool.tile([128, GROUP, W], BF16, tag="ta")
                bb = tailpool.tile([128, GROUP, W], BF16, tag="tb")
                nc.scalar.activation(a, pp, AF.Abs)
                nc.scalar.activation(bb, pt, AF.Abs)
                dd = tailpool.tile([128, GROUP, W], BF16, tag="td")
                nc.vector.tensor_sub(dd, a, bb)
                idx = (ci - 1) * (HS // GROUP) + g
                if STAGE >= 5:
                    # fused square + free-dim-sum on DVE:
                    # out = (dd bypass 1.0) mult dd = dd^2; accum = sum(out)
                    sq = tailpool.tile([128, GROUP, W], BF16, tag="ts")
                    nc.vector.scalar_tensor_tensor(
                        out=sq,
                        in0=dd,
                        scalar=1.0,
                        in1=dd,
                        op0=OP.mult,
                        op1=OP.mult,
                        accum_out=slots[:, idx:idx + 1],
                    )
                else:
                    sq = tailpool.tile([128, GROUP, W], BF16, tag="ts")
                    nc.scalar.activation(
                        sq, dd, AF.Square, accum_out=slots[:, idx:idx + 1]
                    )

        # host reduces the [128, N_SLOTS] partials
        nc.sync.dma_start(out=out_d, in_=slots)


def build_nc():
    nc = bacc.Bacc("TRN2", target_bir_lowering=False, debug=False)
    with tile.TileContext(nc) as tc:
        _emit(tc)
    nc.compile()
    return nc


_CACHE = {}


def _get_nc():
    if "nc" not in _CACHE:
        _CACHE["nc"] = build_nc()
    return _CACHE["nc"]


def make_in_maps(logits, targets):
    """Host-side marshaling: H-pad, W-guard, slice per core, dtype-cast."""
    logits = np.asarray(logits, dtype=np.float32)
    targets = np.asarray(targets)
    # pad H by one row on each side: class0 logit 0, classes 1..3 -> NEG so
    # softmax probs vanish there (matches the reference's zero-padded conv on
    # probs).  W guard columns logit 0 (probs there are killed by rbf=0).
    lp = np.zeros((B, C, D, H + 2, WG), dtype=np.float32)
    lp[:, 1:, :, 0, :] = NEG
    lp[:, 1:, :, H + 1, :] = NEG
    lp[:, :, :, 1:H + 1, 0:W] = logits
    lp = lp.astype(ml_dtypes.bfloat16)
    # targets pad/guard = 255 -> one-hot masks vanish there
    tp = np.full((B, D, H + 2, WG), 255.0, dtype=np.float32)
    tp[:, :, 1:H + 1, 0:W] = targets.astype(np.float32)
    tp = tp.astype(ml_dtypes.bfloat16)

    # interleave partitions: p = 2*d + b
    lp = lp.transpose(1, 2, 0, 3, 4).reshape(C, 2 * D, H + 2, WG)
    tp = tp.transpose(1, 0, 2, 3).reshape(2 * D, H + 2, WG)

    wtd, wi = _stationaries()
    in_maps = []
    for k in range(NCORES):
        h0 = k * HS
        in_maps.append({
            "logits": np.ascontiguousarray(lp[:, :, h0:h0 + HL, :]),
            "targets": np.ascontiguousarray(tp[:, h0:h0 + HL, :]),
            "wTD": wtd,
            "wI": wi,
        })
    return in_maps


def kernel(logits, targets):
    nc = _get_nc()
    in_maps = make_in_maps(logits, targets)
    results = run_bass_kernel_spmd(nc, in_maps, core_ids=list(range(NCORES)))
    total = 0.0
    for r in results.results:
        total += np.asarray(r["out"], dtype=np.float64).sum()
    return np.float32(total / NTOT)


# revision 24
# speedup vs baseline: 1.0466x; 1.0466x over previous
"""Trainium2 Bass kernel for nn_BoundaryLoss (3D-Laplacian boundary loss).

reference semantics (fp32):
    probs = softmax(logits, axis=1)[:, 1:]                  # (B, C-1, D, H, W)
    tmask = one_hot(targets)[classes 1..C-1]                # (B, C-1, D, H, W)
    loss  = mean((|lap3(probs)| - |lap3(tmask)|)**2)        # lap3 = 6-neighbour
                                                            # Laplacian, zero pad

Distribution: pure data parallelism over H (256 rows -> 8 slices of 32 rows,
plus one halo row on each side).  Each core computes the squared-error sum
over its slice; the host adds the 8 partial sums and divides by the global
element count.

On-core layout: SBUF partitions = (b, d) = 2*64 = 128.  Free dim = (h, w') with
w' = W+2 (two zero "guard" columns per row so the w+-1 stencil shifts read
zeros across row boundaries).  Engine assignment (engine-balanced; the
baseline's gpsimd masks at 131us/op serialized the whole kernel):
  - ScalarE: exp (chunked for pipelining), |.| of the two laplacians
  - VectorE: softmax reciprocal (reciprocal_approx_fast), probs mult,
    one-hot masks (tensor_scalar is_equal), diff, fused square+reduce
  - TensorE: softmax denominator accumulation + the 7-point laplacians
    (d+-1/-6 via block-tridiagonal stationary; h/w shifts via offset
    identity-stationary matmuls), PSUM fp32 accumulation
"""

import numpy as np
import ml_dtypes

import concourse.bass as bass
import concourse.bacc as bacc
import concourse.tile as tile
from concourse import mybir
from concourse.bass_utils import run_bass_kernel_spmd

# Problem shape (hardcoded; harness contract)
B, C, D, H, W = 2, 4, 64, 256, 256
NCORES = 8
HS = H // NCORES        # 32 output rows per core
HL = HS + 2             # 34 input rows (1 halo row each side)
WG = W + 2              # guarded row stride in SBUF free dim
GROUP = 4               # output rows per PSUM group (2 banks)
NEG = -100.0            # pad value for classes 1..3 -> softmax prob ~ 0
NTOT = B * (C - 1) * D * H * W  # mean denominator

F32 = mybir.dt.float32
BF16 = mybir.dt.bfloat16
AX = mybir.AxisListType
OP = mybir.AluOpType
AF = mybir.ActivationFunctionType

N_SLOTS = 3 * (HS // GROUP)  # one accumulator slot per (class, group)
import os as _os
STAGE = int(_os.environ.get("K_STAGE", "5"))  # debug bisect: 1..5
# row chunks for exp / probs (pipelining: denominator groups only need their
# own rows, so the PE can start ~8 rows in instead of after 4 whole-class exps)
CHUNKS = [(0, 8), (8, 16), (16, 24), (24, 32), (32, 34)]


def _stationaries():
    """T_D: d-stencil (d+-1 within the same b, -6 on the diagonal) on the
    interleaved partition layout p = 2*d + b.  wI: identity.  Exact in bf16."""
    td = np.zeros((128, 128), dtype=np.float32)
    for p in range(128):
        td[p, p] = -6.0
        d, b = divmod(p, 2)
        if d > 0:
            td[p - 2, p] = 1.0
        if d < D - 1:
            td[p + 2, p] = 1.0
    ident = np.eye(128, dtype=np.float32)
    return (td.astype(ml_dtypes.bfloat16), ident.astype(ml_dtypes.bfloat16))


def _emit(tc):
    nc = tc.nc
    # host pre-interleaves to partition order p = 2*d + b, so every DMA is a
    # plain 2D full-partition transfer
    lg = nc.dram_tensor("logits", [C, 128, HL, WG], BF16, kind="ExternalInput").ap()
    tg = nc.dram_tensor("targets", [128, HL, WG], BF16, kind="ExternalInput").ap()
    wtd_d = nc.dram_tensor("wTD", [128, 128], BF16, kind="ExternalInput").ap()
    wi_d = nc.dram_tensor("wI", [128, 128], BF16, kind="ExternalInput").ap()
    out_d = nc.dram_tensor("out", [128, N_SLOTS], F32, kind="ExternalOutput").ap()

    with (
        tc.tile_pool(name="singles", bufs=1) as singles,
        tc.tile_pool(name="mpool", bufs=3) as mpool,
        tc.tile_pool(name="rpool", bufs=2) as rpool,
        tc.tile_pool(name="tailpool", bufs=3) as tailpool,
        tc.tile_pool(name="psum", bufs=4, space="PSUM") as psum,
    ):
        # --- constants / persistent tiles ---
        wtd = singles.tile([128, 128], BF16, tag="wtd")
        wi = singles.tile([128, 128], BF16, tag="wi")
        nc.sync.dma_start(out=wtd, in_=wtd_d)
        nc.sync.dma_start(out=wi, in_=wi_d)
        slots = singles.tile([128, N_SLOTS], F32, tag="slots")
        nc.vector.memset(slots, 0.0)

        # --- exp of all 4 classes, chunked by rows ---
        # logits arrive bf16, guards 0.0 (-> e guard 1.0, killed by rbf=0),
        # halo rows 0.0 for class 0 / NEG for classes 1..3.
        # DMA + exp are chunked and interleaved class-major so the first
        # denominator matmuls can start ~2us in instead of after 4 whole-class
        # DMAs; exp is done in place on the DMA'd tile.
        # targets: host pads guards/halo with 255 so is_equal(c) vanishes
        # there; DMA chunks trail the logits chunks (mask laps start early)
        tgt = singles.tile([128, HL, WG], BF16, tag="tgt")
        e = []
        for ci in range(C):
            t = singles.tile([128, HL, WG], BF16, tag=f"e{ci}")
            e.append(t)
        for r0, r1 in CHUNKS:
            for ci in range(C):
                nc.sync.dma_start(
                    out=e[ci][:, r0:r1, :], in_=lg[ci][:, r0:r1, :]
                )
                nc.scalar.activation(
                    e[ci][:, r0:r1, :], e[ci][:, r0:r1, :], AF.Exp
                )
            nc.sync.dma_start(out=tgt[:, r0:r1, :], in_=tg[:, r0:r1, :])

        # --- denominator (PE accumulate) -> reciprocal (DVE) -> rbf bf16 ---
        rbf = singles.tile([128, HL, WG], BF16, tag="rbf")
        nc.vector.memset(rbf[:, :, W:WG], 0.0)  # kills probs guards
        for h0 in range(0, HL, GROUP) if STAGE >= 2 else []:
            nr = min(GROUP, HL - h0)
            st = psum.tile([128, GROUP, W], F32, tag="ps")
            for ci in range(C):
                for j in range(0, nr, 2):
                    nc.tensor.matmul(
                        out=st[:, j:j + 2, :],
                        lhsT=wi,
                        rhs=e[ci][:, h0 + j:h0 + j + 2, 0:W],
                        start=(ci == 0),
                        stop=(ci == C - 1),
                    )
            sflat = st[:, 0:nr, :].rearrange("p h w -> p (h w)")
            rf = rpool.tile([128, GROUP * W], F32, tag="rf")
            nc.vector.reciprocal_approx_fast(out=rf[:, 0:nr * W], in_=sflat)
            nc.vector.tensor_copy(
                rbf[:, h0:h0 + nr, 0:W],
                rf[:, 0:nr * W].rearrange("p (h w) -> p h w", w=W),
            )

        # --- probs (in place: e_c *= rbf, chunked) and masks, all classes
        # before the lap loops so the DVE work for class c+1 is never queued
        # behind class c's tail ops ---
        m = []
        for ci in range(1, C) if STAGE >= 3 else []:
            for r0, r1 in CHUNKS:
                nc.vector.tensor_tensor(
                    out=e[ci][:, r0:r1, :],
                    in0=e[ci][:, r0:r1, :],
                    in1=rbf[:, r0:r1, :],
                    op=OP.mult,
                )
            mc = mpool.tile([128, HL, WG], BF16, tag="m")
            for r0, r1 in CHUNKS:
                nc.vector.tensor_scalar(
                    mc[:, r0:r1, :], tgt[:, r0:r1, :], float(ci), None,
                    OP.is_equal,
                )
            m.append(mc)

        # --- laplacians (PE) + tail ---
        def lap_matmuls(pt, src, r0):
            """Accumulate the 7-point Laplacian of src rows [r0, r0+GROUP) into
            psum tile pt.  src is a [128, HL, WG] tile; output rows are
            r0..r0+3 (absolute row indices in the tile).  Each matmul reads a
            4-row window slice so Tile sees region deps (laps can start as
            soon as the producing probs/mask chunk lands, not the whole
            tile)."""
            passes = [
                (wtd, 0),       # d+-1 and -6 center
                (wi, -WG),      # h-1
                (wi, WG),       # h+1
                (wi, -1),       # w-1 (guards supply the zero pad)
                (wi, 1),        # w+1
            ]
            for pi, (wmat, sh) in enumerate(passes):
                for j in range(0, GROUP, 2):
                    w0 = r0 + j - 1
                    win = src[:, w0:w0 + 4, :].rearrange("p h w -> p (h w)")
                    off = WG + sh
                    rhs = win[:, off:off + 2 * WG].rearrange(
                        "p (h w) -> p h w", w=WG
                    )[:, :, 0:W]
                    nc.tensor.matmul(
                        out=pt[:, j:j + 2, :],
                        lhsT=wmat,
                        rhs=rhs,
                        start=(pi == 0),
                        stop=(pi == len(passes) - 1),
                    )

        for ci in range(1, C) if STAGE >= 4 else []:
            pc = e[ci]
            mc = m[ci - 1]
            for g in range(HS // GROUP):
                r0 = 1 + g * GROUP
                pp = psum.tile([128, GROUP, W], F32, tag="ps")
                pt = psum.tile([128, GROUP, W], F32, tag="ps")
                lap_matmuls(pp, pc, r0)
                lap_matmuls(pt, mc, r0)
                a = tailpool.tile([128, GROUP, W], BF16, tag="ta")
                bb = tailpool.tile([128, GROUP, W], BF16, tag="tb")
                nc.scalar.activation(a, pp, AF.Abs)
                nc.scalar.activation(bb, pt, AF.Abs)
                dd = tailpool.tile([128, GROUP, W], BF16, tag="td")
                nc.vector.tensor_sub(dd, a, bb)
                idx = (ci - 1) * (HS // GROUP) + g
                if STAGE >= 5:
                    # fused square + free-dim-sum on DVE:
                    # out = (dd bypass 1.0) mult dd = dd^2; accum = sum(out)
                    sq = tailpool.tile([128, GROUP, W], BF16, tag="ts")
                    nc.vector.scalar_tensor_tensor(
                        out=sq,
                        in0=dd,
                        scalar=1.0,
                        in1=dd,
                        op0=OP.mult,
                        op1=OP.mult,
                        accum_out=slots[:, idx:idx + 1],
                    )
                else:
                    sq = tailpool.tile([128, GROUP, W], BF16, tag="ts")
                    nc.scalar.activation(
                        sq, dd, AF.Square, accum_out=slots[:, idx:idx + 1]
                    )

        # host reduces the [128, N_SLOTS] partials
        nc.sync.dma_start(out=out_d, in_=slots)


def build_nc():
    nc = bacc.Bacc("TRN2", target_bir_lowering=False, debug=False)
    with tile.TileContext(nc) as tc:
        _emit(tc)
    nc.compile()
    return nc


_CACHE = {}


def _get_nc():
    if "nc" not in _CACHE:
        _CACHE["nc"] = build_nc()
    return _CACHE["nc"]


def make_in_maps(logits, targets):
    """Host-side marshaling: H-pad, W-guard, slice per core, dtype-cast."""
    logits = np.asarray(logits, dtype=np.float32)
    targets = np.asarray(targets)
    # pad H by one row on each side: class0 logit 0, classes 1..3 -> NEG so
    # softmax probs vanish there (matches the reference's zero-padded conv on
    # probs).  W guard columns logit 0 (probs there are killed by rbf=0).
    lp = np.zeros((B, C, D, H + 2, WG), dtype=np.float32)
    lp[:, 1:, :, 0, :] = NEG
    lp[:, 1:, :, H + 1, :] = NEG
    lp[:, :, :, 1:H + 1, 0:W] = logits
    lp = lp.astype(ml_dtypes.bfloat16)
    # targets pad/guard = 255 -> one-hot masks vanish there
    tp = np.full((B, D, H + 2, WG), 255.0, dtype=np.float32)
    tp[:, :, 1:H + 1, 0:W] = targets.astype(np.float32)
    tp = tp.astype(ml_dtypes.bfloat16)

    # interleave partitions: p = 2*d + b
    lp = lp.transpose(1, 2, 0, 3, 4).reshape(C, 2 * D, H + 2, WG)
    tp = tp.transpose(1, 0, 2, 3).reshape(2 * D, H + 2, WG)

    wtd, wi = _stationaries()
    in_maps = []
    for k in range(NCORES):
        h0 = k * HS
        in_maps.append({
            "logits": np.ascontiguousarray(lp[:, :, h0:h0 + HL, :]),
            "targets": np.ascontiguousarray(tp[:, h0:h0 + HL, :]),
            "wTD": wtd,
            "wI": wi,
        })
    return in_maps


def kernel(logits, targets):
    nc = _get_nc()
    in_maps = make_in_maps(logits, targets)
    results = run_bass_kernel_spmd(nc, in_maps, core_ids=list(range(NCORES)))
    total = 0.0
    for r in results.results:
        total += np.asarray(r["out"], dtype=np.float64).sum()
    return np.float32(total / NTOT)
